# revision 2
# baseline (speedup 1.0000x reference)
"""Trainium2 Bass kernel for nn_BlockDiagonalLinear_text (hyperbolic block-diag linear).

Math: the reference's per-row operations are all scalar row-scalings, so
  out = alpha_row * y   with  y = x @ blockdiag(W_1..W_16).T
where alpha_row is a chain of tanh/artanh/sqrt scalars of ||x_row|| and
||y_row||.  (The expmap scale s cancels out of res_c except through
saturated tanh arguments - validated numerically against the reference.)

Sharding: data-parallel over rows. 8192 rows -> 8 cores x 1024 rows.
Weights (4 MB + identity) replicated. Per-core kernel streams 8 tiles of
128 rows:
  DMA x tile -> ACT x^2 row-sums -> PE transpose x (128x128 blocks) ->
  fp32r (FP22 single-pass) block matmuls -> DVE copy y to SBUF ->
  ACT y^2 row-sums -> per-row scalar chain ([128,1] ops) ->
  DVE scale y in place -> DMA out.

Uses bacc.Bacc (not raw bass.Bass): its compile() pass legalizes
semaphore waits for the 1-wait-per-instruction TPB ISA (EVSEM splitting,
matmul-wait relocation to LDWEIGHTS).
"""
import sys
import numpy as np

for _p in ("/opt/trn_rl_repo", "/root/.axon_site/_ro/trn_rl_repo"):
    if _p not in sys.path:
        sys.path.append(_p)

import concourse.bass as bass
import concourse.bacc as bacc
import concourse.mybir as mybir
from concourse import tile
from concourse.bass_utils import run_bass_kernel_spmd

R, BS = 16, 256           # 16 diagonal blocks of 256x256
D = R * BS                # 4096
P = 128                   # partitions
N_CORES = 8
ROWS_TOTAL = 4 * 2048     # 8192
ROWS_CORE = ROWS_TOTAL // N_CORES   # 1024
NT = ROWS_CORE // P       # 8 tiles of 128 rows per core
WCOLS = 2 * R * BS        # 8192 weight columns
WIDC = WCOLS + P          # + identity columns

f32 = mybir.dt.float32
f32r = mybir.dt.float32r
AF = mybir.ActivationFunctionType
OP = mybir.AluOpType

CLIP_Z = float(np.float32(1.0) - np.float32(1e-5))          # 0.99999
MAXNORM = float(np.float32(1.0 - 1e-3) / np.float32(0.1))   # 9.99


def build_nc(ablate=()):
    ablate = set(ablate)
    nc = bacc.Bacc()
    # float32r (FP22) end-to-end on the matmul path: walrus requires every
    # producer feeding an fp32r matmul to declare an fp32r output.
    x_d = nc.declare_dram_parameter("x", [ROWS_CORE, D], f32r, isOutput=False)
    w_d = nc.declare_dram_parameter("w", [P, WIDC], f32r, isOutput=False)
    out_d = nc.declare_dram_parameter("out", [ROWS_CORE, D], f32, isOutput=True)

    with tile.TileContext(nc) as tc:
        with (
            tc.tile_pool(name="wpool", bufs=1) as wpool,
            tc.tile_pool(name="xpool", bufs=2) as xpool,
            tc.tile_pool(name="ypool", bufs=3) as ypool,
            tc.tile_pool(name="xtpool", bufs=2) as xtpool,
            tc.tile_pool(name="scrpool", bufs=1) as scrpool,
            tc.tile_pool(name="stats", bufs=2) as stats,
            tc.tile_pool(name="pst", bufs=2, space="PSUM") as pst,
            tc.tile_pool(name="psy", bufs=4, space="PSUM") as psy,
        ):
            w_sb = wpool.tile([P, WIDC], f32r, name="w_sb")
            nc.sync.dma_start(out=w_sb[:], in_=w_d[:])
            id_sb = w_sb[:, WCOLS:WIDC]
            scratch = scrpool.tile([P, D], f32, name="scratch")

            def st(shape, tag):
                return stats.tile(shape, f32, tag=tag, name=tag)

            for i in range(NT):
                x_sb = xpool.tile([P, D], f32r, tag="x", name=f"x_{i}")
                nc.sync.dma_start(out=x_sb[:], in_=x_d[i * P:(i + 1) * P, :])

                q2 = st([P, 2], "q2")
                # qx = sum_k x^2 (row-wise)
                if "squares" not in ablate:
                    nc.scalar.activation(scratch[:], x_sb[:].bitcast(f32),
                                         AF.Square, accum_out=q2[:, 0:1])

                # transpose x tile: xt[:, c*128:+128] = x[:, c*128:+128].T
                xt_sb = xtpool.tile([P, D], f32r, tag="xt", name=f"xt_{i}")
                for c in range(D // P):
                    tp = pst.tile([P, P], f32r, tag="tp", name=f"tp_{i}_{c}")
                    nc.tensor.transpose(tp[:], x_sb[:, c * P:(c + 1) * P], id_sb)
                    nc.vector.tensor_copy(xt_sb[:, c * P:(c + 1) * P], tp[:])

                # block matmuls: y[:, r*256:+256] = x_blk_r @ W_r.T  (fp32r)
                y_sb = ypool.tile([P, D], f32, tag="y", name=f"y_{i}")
                for r in range(R):
                    py = psy.tile([P, BS], f32, tag="py", name=f"py_{i}_{r}")
                    for c in range(2):
                        kc = 2 * r + c
                        nc.tensor.matmul(
                            py[:],
                            xt_sb[:, kc * P:(kc + 1) * P],
                            w_sb[:, kc * BS:(kc + 1) * BS],
                            start=(c == 0), stop=(c == 1),
                        )
                    nc.vector.tensor_copy(y_sb[:, r * BS:(r + 1) * BS], py[:])

                # qy = sum_j y^2 (row-wise)
                if "squares" not in ablate:
                    nc.scalar.activation(scratch[:], y_sb[:], AF.Square,
                                         accum_out=q2[:, 1:2])

                # ---- per-row scalar chain ([128,1] / [128,2] ops) ----
                V = nc.vector
                if "chain" in ablate:
                    alm = st([P, 1], "alm")
                    V.tensor_scalar_mul(alm[:], q2[:, 1:2], 1.0)
                    if "scale" not in ablate:
                        V.tensor_scalar(out=y_sb[:], in0=y_sb[:], scalar1=alm[:],
                                        scalar2=5.0, op0=OP.mult, op1=OP.mult)
                    nc.sync.dma_start(out=out_d[i * P:(i + 1) * P, :], in_=y_sb[:])
                    continue
                lnq = st([P, 2], "lnq")
                nc.scalar.activation(lnq[:], q2[:], AF.Ln)
                U = st([P, 2], "U")   # [u | y_n] = sqrt via exp(0.5 ln q)
                nc.scalar.activation(U[:], lnq[:], AF.Exp, scale=0.5)

                uc = st([P, 1], "uc")
                V.tensor_scalar_max(uc[:], U[:, 0:1], 1e-5)
                t1 = st([P, 1], "t1")
                V.tensor_scalar_mul(t1[:], uc[:], 0.1)
                r1 = st([P, 1], "r1")
                V.reciprocal(r1[:], t1[:])
                args_ = st([P, 1], "args_")
                V.tensor_scalar_min(args_[:], t1[:], 15.0)
                Es = st([P, 1], "Es")
                nc.scalar.activation(Es[:], args_[:], AF.Exp, scale=2.0)
                e1 = st([P, 1], "e1")
                V.tensor_scalar_add(e1[:], Es[:], 1.0)
                r2 = st([P, 1], "r2")
                V.reciprocal(r2[:], e1[:])
                tsx = st([P, 1], "tsx")   # tanh(0.1 u_c)
                V.tensor_scalar(out=tsx[:], in0=r2[:], scalar1=-2.0, scalar2=1.0,
                                op0=OP.mult, op1=OP.add)
                za = st([P, 1], "za")
                V.tensor_scalar_min(za[:], tsx[:], CLIP_Z)
                L = st([P, 2], "L")
                V.tensor_scalar_add(L[:, 0:1], za[:], 1.0)
                V.tensor_scalar(out=L[:, 1:2], in0=za[:], scalar1=-1.0, scalar2=1.0,
                                op0=OP.mult, op1=OP.add)
                lnL = st([P, 2], "lnL")
                nc.scalar.activation(lnL[:], L[:], AF.Ln)
                d_ = st([P, 1], "d_")     # 2*artanh(za)
                V.tensor_sub(d_[:], lnL[:, 0:1], lnL[:, 1:2])
                yns = st([P, 1], "yns")   # y_n clamped for safe reciprocal
                V.tensor_scalar_max(yns[:], U[:, 1:2], 1e-20)
                w1 = st([P, 1], "w1")
                V.tensor_mul(w1[:], U[:, 1:2], r1[:])
                w2 = st([P, 1], "w2")
                V.tensor_mul(w2[:], w1[:], d_[:])
                argt = st([P, 1], "argt")
                V.tensor_scalar(out=argt[:], in0=w2[:], scalar1=0.05, scalar2=15.0,
                                op0=OP.mult, op1=OP.min)
                Et = st([P, 1], "Et")
                nc.scalar.activation(Et[:], argt[:], AF.Exp, scale=2.0)
                e2 = st([P, 1], "e2")
                V.tensor_scalar_add(e2[:], Et[:], 1.0)
                r3 = st([P, 1], "r3")
                V.reciprocal(r3[:], e2[:])
                ttx = st([P, 1], "ttx")   # tanh(arg_t)
                V.tensor_scalar(out=ttx[:], in0=r3[:], scalar1=-2.0, scalar2=1.0,
                                op0=OP.mult, op1=OP.add)
                nrm = st([P, 1], "nrm")
                V.tensor_scalar(out=nrm[:], in0=ttx[:], scalar1=10.0, scalar2=1e-5,
                                op0=OP.mult, op1=OP.max)
                ryn = st([P, 1], "ryn")
                V.reciprocal(ryn[:], yns[:])
                gs = st([P, 1], "gs")
                V.tensor_mul(gs[:], ttx[:], ryn[:])
                rn = st([P, 1], "rn")
                V.reciprocal(rn[:], nrm[:])
                p9 = st([P, 1], "p9")
                V.tensor_scalar_mul(p9[:], rn[:], MAXNORM)
                pf = st([P, 1], "pf")
                V.tensor_scalar_min(pf[:], p9[:], 1.0)
                m_ = st([P, 1], "m_")
                V.tensor_scalar_min(m_[:], nrm[:], MAXNORM)
                zb = st([P, 1], "zb")
                V.tensor_scalar_mul(zb[:], m_[:], 0.1)
                B = st([P, 2], "B")
                V.tensor_scalar_add(B[:, 0:1], zb[:], 1.0)
                V.tensor_scalar(out=B[:, 1:2], in0=zb[:], scalar1=-1.0, scalar2=1.0,
                                op0=OP.mult, op1=OP.add)
                lnB = st([P, 2], "lnB")
                nc.scalar.activation(lnB[:], B[:], AF.Ln)
                db = st([P, 1], "db")     # 2*artanh(0.1 m)
                V.tensor_sub(db[:], lnB[:, 0:1], lnB[:, 1:2])
                rzb = st([P, 1], "rzb")
                V.reciprocal(rzb[:], zb[:])
                a1 = st([P, 1], "a1")
                V.tensor_mul(a1[:], gs[:], pf[:])
                a2 = st([P, 1], "a2")
                V.tensor_mul(a2[:], db[:], rzb[:])
                al = st([P, 1], "al")
                V.tensor_mul(al[:], a1[:], a2[:])
                mask = st([P, 1], "mask")
                V.tensor_scalar(out=mask[:], in0=q2[:, 1:2], scalar1=0.0, scalar2=None,
                                op0=OP.is_gt)
                alm = st([P, 1], "alm")
                V.tensor_mul(alm[:], al[:], mask[:])

                # out = y * alpha * 5  (5 = 10 from gs x 0.5 from artanh halves)
                if "scale" not in ablate:
                    V.tensor_scalar(out=y_sb[:], in0=y_sb[:], scalar1=alm[:],
                                    scalar2=5.0, op0=OP.mult, op1=OP.mult)
                nc.sync.dma_start(out=out_d[i * P:(i + 1) * P, :], in_=y_sb[:])
    nc.finalize()   # Bacc.compile(): reg alloc + EVSEM wait legalization
    return nc


_NC = None


def _get_nc():
    global _NC
    if _NC is None:
        _NC = build_nc()
    return _NC


def _round_fp22(a: np.ndarray) -> np.ndarray:
    # round-to-nearest-even to 13-bit mantissa (float32r / FP22)
    u = a.astype(np.float32).view(np.uint32)
    keep = np.uint32(0xFFFFFC00)
    low = u & np.uint32(0x3FF)
    half = np.uint32(0x200)
    lsb = (u >> np.uint32(10)) & np.uint32(1)
    round_up = (low > half) | ((low == half) & (lsb == 1))
    u = (u & keep) + (round_up.astype(np.uint32) << np.uint32(10))
    return u.view(np.float32)


def _prep_weights(weights: np.ndarray) -> np.ndarray:
    # w_sb[:, (2r+c)*256:+256][p, j] = W[r, j, k=c*128+p]; identity appended.
    wt = (weights.astype(np.float32).transpose(0, 2, 1)      # [r, k, j]
          .reshape(R, 2, P, BS).transpose(2, 0, 1, 3)        # [p, r, c, j]
          .reshape(P, WCOLS))
    return np.ascontiguousarray(
        np.concatenate([_round_fp22(wt), np.eye(P, dtype=np.float32)], axis=1))


def kernel(x: np.ndarray, weights: np.ndarray) -> np.ndarray:
    nc = _get_nc()
    xf = np.ascontiguousarray(x, dtype=np.float32).reshape(ROWS_TOTAL, D)
    wid = _prep_weights(np.asarray(weights))
    in_maps = [
        {"x": xf[i * ROWS_CORE:(i + 1) * ROWS_CORE], "w": wid}
        for i in range(N_CORES)
    ]
    res = run_bass_kernel_spmd(nc, in_maps, list(range(N_CORES)))
    out = np.concatenate([res.results[i]["out"] for i in range(N_CORES)], axis=0)
    return out.reshape(x.shape).astype(np.float32, copy=False)


def _in_maps(x, weights):
    xf = np.ascontiguousarray(x, dtype=np.float32).reshape(ROWS_TOTAL, D)
    wid = _prep_weights(np.asarray(weights))
    return [
        {"x": xf[i * ROWS_CORE:(i + 1) * ROWS_CORE], "w": wid}
        for i in range(N_CORES)
    ]


def run_traced(x, weights, trace_dir):
    """test.py only: run with NTFF tracing, artifacts into trace_dir."""
    return run_bass_kernel_spmd(
        _get_nc(), _in_maps(x, weights), list(range(N_CORES)),
        trace=True, tmpdir=trace_dir)


if __name__ == "__main__":
    xs = np.random.randn(4, 2048, D).astype(np.float32)
    ws = (np.broadcast_to(np.eye(BS, dtype=np.float32), (R, BS, BS))
          + 0.02 * np.random.randn(R, BS, BS).astype(np.float32))
    o = kernel(xs, ws)
    print("kernel ran, out shape", o.shape, o.dtype)



# revision 11
# speedup vs baseline: 1.8951x; 1.8951x over previous
"""Trainium2 Bass kernel for nn_BlockDiagonalLinear_text (hyperbolic block-diag linear).

Math: every per-row op in the reference is a scalar row-scaling, so
  out = alpha_row * y,   y = x @ blockdiag(W_1..W_16).T
and the whole tanh/artanh chain collapses exactly (monotonicity:
artanh(clip(tanh(t), max=z)) == clip(t, max=artanh(z))) to
  alpha = min(1, A/||x||, B/||y||) * [||y||>0]
  A = 10*artanh(1-1e-5), B = 10*artanh(0.999)
(validated numerically against the reference to ~1e-5 in f64).

Sharding: data-parallel over rows; 8192 rows -> 8 cores x 1024 rows;
weights replicated.

Per-core kernel, per 128-row tile (all matmul data bf16):
  host supplies x^T pre-transposed/cast per tile ([tile, k, kchunk, row],
  8KB contiguous per partition -> full-rate DMA). PE per tile: 32-chunk
  Gram accumulation (diag = ||x_row||^2) then 32 block matmuls (the PE
  cannot keep two multi-instruction accumulation groups open at once).
  PSUM y copies cast to bf16 (1 ACT + 3 DVE); ||y||^2 = one big ACT
  Square-with-accum over SBUF y (per-PSUM-chunk on the last tile to
  shorten the tail); 5-op DVE min() chain -> alpha; in-place bf16 scale;
  outputs DMA'd on the GPSIMD SWDGE ring (inputs on SP, weights on ACT).
  sqrt/square/copy live in one ACT table set: no table thrash.
"""
import sys
import numpy as np
import ml_dtypes

for _p in ("/opt/trn_rl_repo", "/root/.axon_site/_ro/trn_rl_repo"):
    if _p not in sys.path:
        sys.path.append(_p)

import concourse.bass as bass
import concourse.bacc as bacc
import concourse.mybir as mybir
from concourse import tile
from concourse.bass_utils import run_bass_kernel_spmd

R, BS = 16, 256           # 16 diagonal blocks of 256x256
D = R * BS                # 4096
P = 128                   # partitions
NCH = D // P              # 32 contraction chunks of 128
N_CORES = 8
ROWS_TOTAL = 4 * 2048     # 8192
ROWS_CORE = ROWS_TOTAL // N_CORES   # 1024
NT = ROWS_CORE // P       # 8 tiles of 128 rows per core
WCOLS = D * 2             # 8192 weight cols: chunk kc -> [k_local, j(256)]

f32 = mybir.dt.float32
bf16 = mybir.dt.bfloat16
AF = mybir.ActivationFunctionType
OP = mybir.AluOpType
bfnp = ml_dtypes.bfloat16

# alpha = min(1, A61/||x||, B38/||y||): exact collapse of the reference's
# expmap/mobius/project/logmap chain (f32 clip constants).
_CLIP1 = float(np.float32(1.0) - np.float32(1e-5))            # 0.99999
_MAXN = float(np.float32(1.0 - 1e-3) / np.float32(0.1))       # 9.99
_CLIP2 = float(np.float32(0.1) * np.float32(_MAXN))           # 0.999
A61 = float(10.0 * np.arctanh(np.float64(_CLIP1)))            # 61.0303...
B38 = float(10.0 * np.arctanh(np.float64(_CLIP2)))            # 38.0020...


def build_nc():
    nc = bacc.Bacc()
    xt_d = nc.declare_dram_parameter("xt", [NT, P, NCH, P], bf16,
                                     isOutput=False)
    w_d = nc.declare_dram_parameter("w", [P, WCOLS], bf16, isOutput=False)
    idm_d = nc.declare_dram_parameter("idm", [P, P], f32, isOutput=False)
    out_d = nc.declare_dram_parameter("out", [ROWS_CORE, D], bf16,
                                      isOutput=True)

    with tile.TileContext(nc) as tc:
        with (
            tc.tile_pool(name="wpool", bufs=1) as wpool,
            tc.tile_pool(name="xpool", bufs=3) as xpool,
            tc.tile_pool(name="ypool", bufs=2) as ypool,
            tc.tile_pool(name="scr", bufs=2) as scr,
            tc.tile_pool(name="stats", bufs=3) as stats,
            tc.tile_pool(name="psg", bufs=2, space="PSUM") as psg,
            tc.tile_pool(name="psy", bufs=3, space="PSUM") as psy,
        ):
            V = nc.vector
            w_sb = wpool.tile([P, WCOLS], bf16, name="w_sb")
            idm = wpool.tile([P, P], f32, name="idm")
            nc.sync.dma_start(out=idm[:], in_=idm_d[:])
            # weights on the ACT HWDGE ring, xt tiles on the SP ring
            nc.scalar.dma_start(out=w_sb[:], in_=w_d[:])

            for i in range(NT):
                xt = xpool.tile([P, NCH, P], bf16, tag="xt", name=f"xt_{i}")
                nc.sync.dma_start(out=xt[:], in_=xt_d[i])

                # ---- PE: gram phase, then block-matmul phase ----
                gram = psg.tile([P, P], f32, tag="gram", name=f"g_{i}")
                for kc in range(NCH):
                    nc.tensor.matmul(gram[:], xt[:, kc, :], xt[:, kc, :],
                                     start=(kc == 0), stop=(kc == NCH - 1))
                pys = [psy.tile([P, 1024], f32, tag="py", name=f"py_{i}_{g}")
                       for g in range(4)]
                for kc in range(NCH):
                    r = kc // 2
                    g, q = r // 4, r % 4
                    nc.tensor.matmul(
                        pys[g][:, q * BS:(q + 1) * BS],
                        xt[:, kc, :], w_sb[:, kc * BS:(kc + 1) * BS],
                        start=(kc % 2 == 0), stop=(kc % 2 == 1),
                    )

                # ---- qx = diag(gram) via identity mask + row-reduce ----
                st2 = stats.tile([P, 2], f32, tag="st2", name=f"st2_{i}")
                dsc = scr.tile([P, P], f32, tag="dsc", name=f"dsc_{i}")
                V.tensor_tensor(out=dsc[:], in0=gram[:], in1=idm[:],
                                op=OP.mult)
                V.tensor_reduce(st2[:, 0:1], dsc[:],
                                axis=mybir.AxisListType.X, op=OP.add)

                # ---- PSUM -> SBUF copies (cast bf16): 1 ACT + 3 DVE ----
                y_sb = ypool.tile([P, D], bf16, tag="y", name=f"y_{i}")
                for g in range(4):
                    ysl = y_sb[:, g * 1024:(g + 1) * 1024]
                    if g == 0:
                        nc.scalar.copy(ysl, pys[g][:])
                    else:
                        V.tensor_copy(ysl, pys[g][:])

                # ---- qy = ||y||^2 ----
                sq = scr.tile([P, D], bf16, tag="sq", name=f"sq_{i}")
                if i < NT - 1:
                    # one big Square+accum over SBUF y (fewest ACT ops)
                    nc.scalar.activation(sq[:], y_sb[:], AF.Square,
                                         accum_out=st2[:, 1:2])
                else:
                    # last tile: per-PSUM-chunk partials for a short tail
                    qyp = stats.tile([P, 4], f32, tag="qyp", name=f"qyp_{i}")
                    for g in range(4):
                        nc.scalar.activation(sq[:, 0:1024], pys[g][:],
                                             AF.Square,
                                             accum_out=qyp[:, g:g + 1])
                    V.tensor_reduce(st2[:, 1:2], qyp[:],
                                    axis=mybir.AxisListType.X, op=OP.add)

                # ---- alpha = min(1, A61/u, B38/yn) * [qy>0] ----
                mask = stats.tile([P, 1], f32, tag="mask", name=f"mk_{i}")
                V.tensor_scalar(out=mask[:], in0=st2[:, 1:2], scalar1=0.0,
                                scalar2=None, op0=OP.is_gt)
                s2 = stats.tile([P, 2], f32, tag="s2", name=f"s2_{i}")
                nc.scalar.activation(s2[:], st2[:], AF.Sqrt)
                uc = stats.tile([P, 2], f32, tag="uc", name=f"uc_{i}")
                V.tensor_scalar_max(uc[:], s2[:], 1e-5)
                rc = stats.tile([P, 2], f32, tag="rc", name=f"rc_{i}")
                V.reciprocal(rc[:], uc[:])
                ta = stats.tile([P, 1], f32, tag="ta", name=f"ta_{i}")
                V.tensor_scalar_mul(ta[:], rc[:, 0:1], A61)
                al0 = stats.tile([P, 1], f32, tag="al0", name=f"al0_{i}")
                V.scalar_tensor_tensor(out=al0[:], in0=rc[:, 1:2],
                                       scalar=B38, in1=ta[:],
                                       op0=OP.mult, op1=OP.min)
                alm = stats.tile([P, 1], f32, tag="alm", name=f"alm_{i}")
                V.scalar_tensor_tensor(out=alm[:], in0=al0[:], scalar=1.0,
                                       in1=mask[:], op0=OP.min, op1=OP.mult)

                # ---- in-place bf16 scale (4x DVE mode), SWDGE out-DMA ----
                V.tensor_scalar(out=y_sb[:], in0=y_sb[:], scalar1=alm[:],
                                scalar2=None, op0=OP.mult)
                nc.gpsimd.dma_start(out=out_d[i * P:(i + 1) * P, :],
                                    in_=y_sb[:])
    nc.finalize()
    return nc


_NC = None


def _get_nc():
    global _NC
    if _NC is None:
        _NC = build_nc()
    return _NC


def _prep_weights(weights: np.ndarray) -> np.ndarray:
    # w_sb[p, kc*256 + j] = W[r, j, k], k = kc*128 + p, kc = 2r + c
    wt = (weights.astype(np.float32).transpose(0, 2, 1)      # [r, k, j]
          .reshape(R, 2, P, BS).transpose(2, 0, 1, 3)        # [p, r, c, j]
          .reshape(P, WCOLS))
    return np.ascontiguousarray(wt.astype(bfnp))


def _in_maps(x, weights):
    xf = np.ascontiguousarray(x, dtype=np.float32).reshape(ROWS_TOTAL, D)
    xb = xf.astype(bfnp)
    wid = _prep_weights(np.asarray(weights))
    idm = np.eye(P, dtype=np.float32)
    maps = []
    for c in range(N_CORES):
        xc = xb[c * ROWS_CORE:(c + 1) * ROWS_CORE]           # [1024, 4096]
        # xt[t, p, kc, row] = xc[t*128 + row, kc*128 + p]
        xt = np.ascontiguousarray(
            xc.reshape(NT, P, NCH, P).transpose(0, 3, 2, 1))
        maps.append({"xt": xt, "w": wid, "idm": idm})
    return maps


def kernel(x: np.ndarray, weights: np.ndarray) -> np.ndarray:
    nc = _get_nc()
    res = run_bass_kernel_spmd(nc, _in_maps(x, weights), list(range(N_CORES)))
    out = np.concatenate(
        [np.asarray(res.results[i]["out"]) for i in range(N_CORES)], axis=0)
    return out.reshape(x.shape).astype(np.float32)


def run_traced(x, weights, trace_dir):
    """test.py only: run with NTFF tracing, artifacts into trace_dir."""
    return run_bass_kernel_spmd(
        _get_nc(), _in_maps(x, weights), list(range(N_CORES)),
        trace=True, tmpdir=trace_dir)


if __name__ == "__main__":
    xs = np.random.randn(4, 2048, D).astype(np.float32)
    ws = (np.broadcast_to(np.eye(BS, dtype=np.float32), (16, BS, BS))
          + 0.02 * np.random.randn(16, BS, BS).astype(np.float32))
    o = kernel(xs, ws)
    print("kernel ran, out shape", o.shape, o.dtype)


# revision 13
# speedup vs baseline: 2.3807x; 1.2562x over previous
"""Trainium2 Bass kernel for nn_BlockDiagonalLinear_text (hyperbolic block-diag linear).

Math: every per-row op in the reference is a scalar row-scaling, so
  out = alpha_row * y,   y = x @ blockdiag(W_1..W_16).T
and the whole tanh/artanh chain collapses exactly (monotonicity:
artanh(clip(tanh(t), max=z)) == clip(t, max=artanh(z))) to
  alpha = min(1, A/||x||, B/||y||) * [||y||>0]
  A = 10*artanh(1-1e-5), B = 10*artanh(0.999)
(validated numerically against the reference to ~1e-5 in f64).

Sharding: data-parallel over rows; 8192 rows -> 8 cores x 1024 rows;
weights replicated.

Per-core kernel, per 128-row tile (all matmul data bf16):
  host supplies x^T pre-transposed/cast per tile ([tile, k, kchunk, row],
  8KB contiguous per partition -> full-rate DMA). PE per tile: 32-chunk
  Gram accumulation (diag = ||x_row||^2) then 32 block matmuls (the PE
  cannot keep two multi-instruction accumulation groups open at once).
  PSUM y copies cast to bf16 (1 ACT + 3 DVE); ||y||^2 = one big ACT
  Square-with-accum over SBUF y (per-PSUM-chunk on the last tile to
  shorten the tail); 5-op DVE min() chain -> alpha; in-place bf16 scale;
  outputs DMA'd on the GPSIMD SWDGE ring (inputs on SP, weights on ACT).
  sqrt/square/copy live in one ACT table set: no table thrash.
"""
import sys
import numpy as np
import ml_dtypes

for _p in ("/opt/trn_rl_repo", "/root/.axon_site/_ro/trn_rl_repo"):
    if _p not in sys.path:
        sys.path.append(_p)

import concourse.bass as bass
import concourse.bacc as bacc
import concourse.mybir as mybir
from concourse import tile
from concourse.bass_utils import run_bass_kernel_spmd

R, BS = 16, 256           # 16 diagonal blocks of 256x256
D = R * BS                # 4096
P = 128                   # partitions
NCH = D // P              # 32 contraction chunks of 128
N_CORES = 8
ROWS_TOTAL = 4 * 2048     # 8192
ROWS_CORE = ROWS_TOTAL // N_CORES   # 1024
NT = ROWS_CORE // P       # 8 tiles of 128 rows per core
WCOLS = D * 2             # 8192 weight cols: chunk kc -> [k_local, j(256)]

f32 = mybir.dt.float32
bf16 = mybir.dt.bfloat16
AF = mybir.ActivationFunctionType
OP = mybir.AluOpType
bfnp = ml_dtypes.bfloat16

# alpha = min(1, A61/||x||, B38/||y||): exact collapse of the reference's
# expmap/mobius/project/logmap chain (f32 clip constants).
_CLIP1 = float(np.float32(1.0) - np.float32(1e-5))            # 0.99999
_MAXN = float(np.float32(1.0 - 1e-3) / np.float32(0.1))       # 9.99
_CLIP2 = float(np.float32(0.1) * np.float32(_MAXN))           # 0.999
A61 = float(10.0 * np.arctanh(np.float64(_CLIP1)))            # 61.0303...
B38 = float(10.0 * np.arctanh(np.float64(_CLIP2)))            # 38.0020...


def build_nc():
    nc = bacc.Bacc()
    xt_d = nc.declare_dram_parameter("xt", [NT, P, NCH, P], bf16,
                                     isOutput=False)
    w_d = nc.declare_dram_parameter("w", [P, WCOLS], bf16, isOutput=False)
    idm_d = nc.declare_dram_parameter("idm", [P, P], f32, isOutput=False)
    out_d = nc.declare_dram_parameter("out", [ROWS_CORE, D], bf16,
                                      isOutput=True)

    with tile.TileContext(nc) as tc:
        with (
            tc.tile_pool(name="wpool", bufs=1) as wpool,
            tc.tile_pool(name="xpool", bufs=2) as xpool,
            tc.tile_pool(name="ypool", bufs=3) as ypool,
            tc.tile_pool(name="opool", bufs=2) as opool,
            tc.tile_pool(name="scr", bufs=2) as scr,
            tc.tile_pool(name="stats", bufs=3) as stats,
            tc.tile_pool(name="psg", bufs=2, space="PSUM") as psg,
            tc.tile_pool(name="psy", bufs=3, space="PSUM") as psy,
        ):
            V = nc.vector
            w_sb = wpool.tile([P, WCOLS], bf16, name="w_sb")
            idm = wpool.tile([P, P], f32, name="idm")
            # weights on the ACT HWDGE ring; xt/idm/out on the SP ring.
            # (concurrent DMAs round-robin per packet across the SDMA
            # engines, so the first tile's load is split into 4 slices -
            # the gram phase starts after the first 256KB - and prefetch
            # depth stays at 2 so tile0 isn't delayed by deep prefetch.)
            nc.scalar.dma_start(out=w_sb[:], in_=w_d[:])

            for i in range(NT):
                xt = xpool.tile([P, NCH, P], bf16, tag="xt", name=f"xt_{i}")
                if i == 0:
                    for s in range(4):
                        nc.sync.dma_start(
                            out=xt[:, s * 8:(s + 1) * 8, :],
                            in_=xt_d[i, :, s * 8:(s + 1) * 8, :])
                    nc.sync.dma_start(out=idm[:], in_=idm_d[:])
                else:
                    nc.sync.dma_start(out=xt[:], in_=xt_d[i])

                # ---- PE: gram phase, then block-matmul phase ----
                gram = psg.tile([P, P], f32, tag="gram", name=f"g_{i}")
                for kc in range(NCH):
                    nc.tensor.matmul(gram[:], xt[:, kc, :], xt[:, kc, :],
                                     start=(kc == 0), stop=(kc == NCH - 1))
                pys = [psy.tile([P, 1024], f32, tag="py", name=f"py_{i}_{g}")
                       for g in range(4)]
                for kc in range(NCH):
                    r = kc // 2
                    g, q = r // 4, r % 4
                    nc.tensor.matmul(
                        pys[g][:, q * BS:(q + 1) * BS],
                        xt[:, kc, :], w_sb[:, kc * BS:(kc + 1) * BS],
                        start=(kc % 2 == 0), stop=(kc % 2 == 1),
                    )

                # ---- qx = diag(gram) via identity mask + row-reduce ----
                st2 = stats.tile([P, 2], f32, tag="st2", name=f"st2_{i}")
                dsc = scr.tile([P, P], f32, tag="dsc", name=f"dsc_{i}")
                V.tensor_tensor(out=dsc[:], in0=gram[:], in1=idm[:],
                                op=OP.mult)
                V.tensor_reduce(st2[:, 0:1], dsc[:],
                                axis=mybir.AxisListType.X, op=OP.add)

                # ---- PSUM -> SBUF copies (cast bf16): 1 ACT + 3 DVE ----
                y_sb = ypool.tile([P, D], bf16, tag="y", name=f"y_{i}")
                for g in range(4):
                    ysl = y_sb[:, g * 1024:(g + 1) * 1024]
                    if g == 0:
                        nc.scalar.copy(ysl, pys[g][:])
                    else:
                        V.tensor_copy(ysl, pys[g][:])

                # ---- qy = ||y||^2 ----
                sq = scr.tile([P, D], bf16, tag="sq", name=f"sq_{i}")
                if i < NT - 1:
                    # one big Square+accum over SBUF y (fewest ACT ops)
                    nc.scalar.activation(sq[:], y_sb[:], AF.Square,
                                         accum_out=st2[:, 1:2])
                else:
                    # last tile: per-PSUM-chunk partials for a short tail
                    qyp = stats.tile([P, 4], f32, tag="qyp", name=f"qyp_{i}")
                    for g in range(4):
                        nc.scalar.activation(sq[:, 0:1024], pys[g][:],
                                             AF.Square,
                                             accum_out=qyp[:, g:g + 1])
                    V.tensor_reduce(st2[:, 1:2], qyp[:],
                                    axis=mybir.AxisListType.X, op=OP.add)

                # ---- alpha = min(1, A61/u, B38/yn) * [qy>0] ----
                mask = stats.tile([P, 1], f32, tag="mask", name=f"mk_{i}")
                V.tensor_scalar(out=mask[:], in0=st2[:, 1:2], scalar1=0.0,
                                scalar2=None, op0=OP.is_gt)
                s2 = stats.tile([P, 2], f32, tag="s2", name=f"s2_{i}")
                nc.scalar.activation(s2[:], st2[:], AF.Sqrt)
                uc = stats.tile([P, 2], f32, tag="uc", name=f"uc_{i}")
                V.tensor_scalar_max(uc[:], s2[:], 1e-5)
                rc = stats.tile([P, 2], f32, tag="rc", name=f"rc_{i}")
                V.reciprocal(rc[:], uc[:])
                ta = stats.tile([P, 1], f32, tag="ta", name=f"ta_{i}")
                V.tensor_scalar_mul(ta[:], rc[:, 0:1], A61)
                al0 = stats.tile([P, 1], f32, tag="al0", name=f"al0_{i}")
                V.scalar_tensor_tensor(out=al0[:], in0=rc[:, 1:2],
                                       scalar=B38, in1=ta[:],
                                       op0=OP.mult, op1=OP.min)
                alm = stats.tile([P, 1], f32, tag="alm", name=f"alm_{i}")
                V.scalar_tensor_tensor(out=alm[:], in0=al0[:], scalar=1.0,
                                       in1=mask[:], op0=OP.min, op1=OP.mult)

                # ---- bf16 scale (4x DVE mode), SP-ring out-DMA ----
                o_sb = opool.tile([P, D], bf16, tag="o", name=f"o_{i}")
                V.tensor_scalar(out=o_sb[:], in0=y_sb[:], scalar1=alm[:],
                                scalar2=None, op0=OP.mult)
                nc.sync.dma_start(out=out_d[i * P:(i + 1) * P, :],
                                  in_=o_sb[:])
    nc.finalize()
    return nc


_NC = None


def _get_nc():
    global _NC
    if _NC is None:
        _NC = build_nc()
    return _NC


def _prep_weights(weights: np.ndarray) -> np.ndarray:
    # w_sb[p, kc*256 + j] = W[r, j, k], k = kc*128 + p, kc = 2r + c
    wt = (weights.astype(np.float32).transpose(0, 2, 1)      # [r, k, j]
          .reshape(R, 2, P, BS).transpose(2, 0, 1, 3)        # [p, r, c, j]
          .reshape(P, WCOLS))
    return np.ascontiguousarray(wt.astype(bfnp))


def _in_maps(x, weights):
    xf = np.ascontiguousarray(x, dtype=np.float32).reshape(ROWS_TOTAL, D)
    xb = xf.astype(bfnp)
    wid = _prep_weights(np.asarray(weights))
    idm = np.eye(P, dtype=np.float32)
    maps = []
    for c in range(N_CORES):
        xc = xb[c * ROWS_CORE:(c + 1) * ROWS_CORE]           # [1024, 4096]
        # xt[t, p, kc, row] = xc[t*128 + row, kc*128 + p]
        xt = np.ascontiguousarray(
            xc.reshape(NT, P, NCH, P).transpose(0, 3, 2, 1))
        maps.append({"xt": xt, "w": wid, "idm": idm})
    return maps


def kernel(x: np.ndarray, weights: np.ndarray) -> np.ndarray:
    nc = _get_nc()
    res = run_bass_kernel_spmd(nc, _in_maps(x, weights), list(range(N_CORES)))
    out = np.concatenate(
        [np.asarray(res.results[i]["out"]) for i in range(N_CORES)], axis=0)
    return out.reshape(x.shape).astype(np.float32)


def run_traced(x, weights, trace_dir):
    """test.py only: run with NTFF tracing, artifacts into trace_dir."""
    return run_bass_kernel_spmd(
        _get_nc(), _in_maps(x, weights), list(range(N_CORES)),
        trace=True, tmpdir=trace_dir)


if __name__ == "__main__":
    xs = np.random.randn(4, 2048, D).astype(np.float32)
    ws = (np.broadcast_to(np.eye(BS, dtype=np.float32), (16, BS, BS))
          + 0.02 * np.random.randn(16, BS, BS).astype(np.float32))
    o = kernel(xs, ws)
    print("kernel ran, out shape", o.shape, o.dtype)


# revision 15
# speedup vs baseline: 2.4453x; 1.0271x over previous
"""Trainium2 Bass kernel for nn_BlockDiagonalLinear_text (hyperbolic block-diag linear).

Math: every per-row op in the reference is a scalar row-scaling, so
  out = alpha_row * y,   y = x @ blockdiag(W_1..W_16).T
and the whole tanh/artanh chain collapses exactly (monotonicity:
artanh(clip(tanh(t), max=z)) == clip(t, max=artanh(z))) to
  alpha = min(1, A/||x||, B/||y||) * [||y||>0]
  A = 10*artanh(1-1e-5), B = 10*artanh(0.999)
(validated numerically against the reference to ~1e-5 in f64).

Sharding: data-parallel over rows; 8192 rows -> 8 cores x 1024 rows;
weights replicated.

Per-core kernel, per 128-row tile (all matmul data bf16):
  host supplies x^T pre-transposed/cast per tile ([tile, k, kchunk, row],
  8KB contiguous per partition -> full-rate DMA). PE per tile: 32-chunk
  Gram accumulation (diag = ||x_row||^2) then 32 block matmuls (the PE
  cannot keep two multi-instruction accumulation groups open at once).
  PSUM y copies cast to bf16 (1 ACT + 3 DVE); ||y||^2 = one big ACT
  Square-with-accum over SBUF y (per-PSUM-chunk on the last tile to
  shorten the tail); 5-op DVE min() chain -> alpha; in-place bf16 scale;
  outputs DMA'd on the GPSIMD SWDGE ring (inputs on SP, weights on ACT).
  sqrt/square/copy live in one ACT table set: no table thrash.
"""
import sys
import numpy as np
import ml_dtypes

for _p in ("/opt/trn_rl_repo", "/root/.axon_site/_ro/trn_rl_repo"):
    if _p not in sys.path:
        sys.path.append(_p)

import concourse.bass as bass
import concourse.bacc as bacc
import concourse.mybir as mybir
from concourse import tile
from concourse.bass_utils import run_bass_kernel_spmd

R, BS = 16, 256           # 16 diagonal blocks of 256x256
D = R * BS                # 4096
P = 128                   # partitions
NCH = D // P              # 32 contraction chunks of 128
N_CORES = 8
ROWS_TOTAL = 4 * 2048     # 8192
ROWS_CORE = ROWS_TOTAL // N_CORES   # 1024
NT = ROWS_CORE // P       # 8 tiles of 128 rows per core
WCOLS = D * 2             # 8192 weight cols: chunk kc -> [k_local, j(256)]

f32 = mybir.dt.float32
bf16 = mybir.dt.bfloat16
AF = mybir.ActivationFunctionType
OP = mybir.AluOpType
bfnp = ml_dtypes.bfloat16

# alpha = min(1, A61/||x||, B38/||y||): exact collapse of the reference's
# expmap/mobius/project/logmap chain (f32 clip constants).
_CLIP1 = float(np.float32(1.0) - np.float32(1e-5))            # 0.99999
_MAXN = float(np.float32(1.0 - 1e-3) / np.float32(0.1))       # 9.99
_CLIP2 = float(np.float32(0.1) * np.float32(_MAXN))           # 0.999
A61 = float(10.0 * np.arctanh(np.float64(_CLIP1)))            # 61.0303...
B38 = float(10.0 * np.arctanh(np.float64(_CLIP2)))            # 38.0020...


def build_nc():
    nc = bacc.Bacc()
    xt_d = nc.declare_dram_parameter("xt", [NT, P, NCH, P], bf16,
                                     isOutput=False)
    w_d = nc.declare_dram_parameter("w", [P, WCOLS], bf16, isOutput=False)
    idm_d = nc.declare_dram_parameter("idm", [P, P], f32, isOutput=False)
    out_d = nc.declare_dram_parameter("out", [ROWS_CORE, D], bf16,
                                      isOutput=True)

    with tile.TileContext(nc) as tc:
        with (
            tc.tile_pool(name="wpool", bufs=1) as wpool,
            tc.tile_pool(name="xpool", bufs=2) as xpool,
            tc.tile_pool(name="ypool", bufs=3) as ypool,
            tc.tile_pool(name="opool", bufs=2) as opool,
            tc.tile_pool(name="scr", bufs=2) as scr,
            tc.tile_pool(name="stats", bufs=3) as stats,
            tc.tile_pool(name="psg", bufs=2, space="PSUM") as psg,
            tc.tile_pool(name="psy", bufs=3, space="PSUM") as psy,
        ):
            V = nc.vector
            w_sb = wpool.tile([P, WCOLS], bf16, name="w_sb")
            idm = wpool.tile([P, P], f32, name="idm")
            # weights on the ACT HWDGE ring; xt/idm/out on the SP ring.
            # (concurrent DMAs round-robin per packet across the SDMA
            # engines, so the first tile's load is split into 4 slices -
            # the gram phase starts after the first 256KB - and prefetch
            # depth stays at 2 so tile0 isn't delayed by deep prefetch.)
            for s in range(4):
                qw = WCOLS // 4
                nc.scalar.dma_start(out=w_sb[:, s * qw:(s + 1) * qw],
                                    in_=w_d[:, s * qw:(s + 1) * qw])

            # one-tile-lag software pipeline: tile i-1's qy/chain/scale/DMA
            # are emitted AFTER tile i's copies so the next tile's g0 copy
            # never queues behind the previous tile's 3.6us Square on ACT.
            prev = None

            def finish(s, last):
                i, y_sb, st2, pys = s
                sq = scr.tile([P, D], bf16, tag="sq", name=f"sq_{i}")
                if not last:
                    # one big Square+accum over SBUF y (fewest ACT ops)
                    nc.scalar.activation(sq[:], y_sb[:], AF.Square,
                                         accum_out=st2[:, 1:2])
                else:
                    # short tail: per-chunk Squares pipeline with copies
                    qyp = stats.tile([P, 4], f32, tag="qyp", name=f"qyp_{i}")
                    for g in range(4):
                        nc.scalar.activation(
                            sq[:, g * 1024:(g + 1) * 1024],
                            y_sb[:, g * 1024:(g + 1) * 1024],
                            AF.Square, accum_out=qyp[:, g:g + 1])
                    V.tensor_reduce(st2[:, 1:2], qyp[:],
                                    axis=mybir.AxisListType.X, op=OP.add)

                # alpha = min(1, A61/u, B38/yn) * [qy>0]
                mask = stats.tile([P, 1], f32, tag="mask", name=f"mk_{i}")
                V.tensor_scalar(out=mask[:], in0=st2[:, 1:2], scalar1=0.0,
                                scalar2=None, op0=OP.is_gt)
                s2 = stats.tile([P, 2], f32, tag="s2", name=f"s2_{i}")
                nc.scalar.activation(s2[:], st2[:], AF.Sqrt)
                uc = stats.tile([P, 2], f32, tag="uc", name=f"uc_{i}")
                V.tensor_scalar_max(uc[:], s2[:], 1e-5)
                rc = stats.tile([P, 2], f32, tag="rc", name=f"rc_{i}")
                V.reciprocal(rc[:], uc[:])
                ta = stats.tile([P, 1], f32, tag="ta", name=f"ta_{i}")
                V.tensor_scalar_mul(ta[:], rc[:, 0:1], A61)
                al0 = stats.tile([P, 1], f32, tag="al0", name=f"al0_{i}")
                V.scalar_tensor_tensor(out=al0[:], in0=rc[:, 1:2],
                                       scalar=B38, in1=ta[:],
                                       op0=OP.mult, op1=OP.min)
                alm = stats.tile([P, 1], f32, tag="alm", name=f"alm_{i}")
                V.scalar_tensor_tensor(out=alm[:], in0=al0[:], scalar=1.0,
                                       in1=mask[:], op0=OP.min, op1=OP.mult)
                # bf16 scale (4x DVE mode), SP-ring out-DMA
                o_sb = opool.tile([P, D], bf16, tag="o", name=f"o_{i}")
                V.tensor_scalar(out=o_sb[:], in0=y_sb[:], scalar1=alm[:],
                                scalar2=None, op0=OP.mult)
                nc.sync.dma_start(out=out_d[i * P:(i + 1) * P, :],
                                  in_=o_sb[:])

            for i in range(NT):
                xt = xpool.tile([P, NCH, P], bf16, tag="xt", name=f"xt_{i}")
                if i == 0:
                    for s in range(4):
                        nc.sync.dma_start(
                            out=xt[:, s * 8:(s + 1) * 8, :],
                            in_=xt_d[i, :, s * 8:(s + 1) * 8, :])
                    nc.sync.dma_start(out=idm[:], in_=idm_d[:])
                else:
                    nc.sync.dma_start(out=xt[:], in_=xt_d[i])

                # ---- PE: gram phase, then block-matmul phase ----
                gram = psg.tile([P, P], f32, tag="gram", name=f"g_{i}")
                for kc in range(NCH):
                    nc.tensor.matmul(gram[:], xt[:, kc, :], xt[:, kc, :],
                                     start=(kc == 0), stop=(kc == NCH - 1))
                pys = [psy.tile([P, 1024], f32, tag="py", name=f"py_{i}_{g}")
                       for g in range(4)]
                for kc in range(NCH):
                    r = kc // 2
                    g, q = r // 4, r % 4
                    nc.tensor.matmul(
                        pys[g][:, q * BS:(q + 1) * BS],
                        xt[:, kc, :], w_sb[:, kc * BS:(kc + 1) * BS],
                        start=(kc % 2 == 0), stop=(kc % 2 == 1),
                    )

                # ---- qx = diag(gram) via identity mask + row-reduce ----
                st2 = stats.tile([P, 2], f32, tag="st2", name=f"st2_{i}")
                dsc = scr.tile([P, P], f32, tag="dsc", name=f"dsc_{i}")
                V.tensor_tensor(out=dsc[:], in0=gram[:], in1=idm[:],
                                op=OP.mult)
                V.tensor_reduce(st2[:, 0:1], dsc[:],
                                axis=mybir.AxisListType.X, op=OP.add)

                # ---- PSUM -> SBUF copies (cast bf16): 1 ACT + 3 DVE ----
                y_sb = ypool.tile([P, D], bf16, tag="y", name=f"y_{i}")
                for g in range(4):
                    ysl = y_sb[:, g * 1024:(g + 1) * 1024]
                    if g == 0:
                        nc.scalar.copy(ysl, pys[g][:])
                    else:
                        V.tensor_copy(ysl, pys[g][:])

                if prev is not None:
                    finish(prev, last=False)
                prev = (i, y_sb, st2, pys)

            finish(prev, last=True)
    nc.finalize()
    return nc


_NC = None


def _get_nc():
    global _NC
    if _NC is None:
        _NC = build_nc()
    return _NC


def _prep_weights(weights: np.ndarray) -> np.ndarray:
    # w_sb[p, kc*256 + j] = W[r, j, k], k = kc*128 + p, kc = 2r + c
    wt = (weights.astype(np.float32).transpose(0, 2, 1)      # [r, k, j]
          .reshape(R, 2, P, BS).transpose(2, 0, 1, 3)        # [p, r, c, j]
          .reshape(P, WCOLS))
    return np.ascontiguousarray(wt.astype(bfnp))


def _in_maps(x, weights):
    xf = np.ascontiguousarray(x, dtype=np.float32).reshape(ROWS_TOTAL, D)
    xb = xf.astype(bfnp)
    wid = _prep_weights(np.asarray(weights))
    idm = np.eye(P, dtype=np.float32)
    maps = []
    for c in range(N_CORES):
        xc = xb[c * ROWS_CORE:(c + 1) * ROWS_CORE]           # [1024, 4096]
        # xt[t, p, kc, row] = xc[t*128 + row, kc*128 + p]
        xt = np.ascontiguousarray(
            xc.reshape(NT, P, NCH, P).transpose(0, 3, 2, 1))
        maps.append({"xt": xt, "w": wid, "idm": idm})
    return maps


def kernel(x: np.ndarray, weights: np.ndarray) -> np.ndarray:
    nc = _get_nc()
    res = run_bass_kernel_spmd(nc, _in_maps(x, weights), list(range(N_CORES)))
    out = np.concatenate(
        [np.asarray(res.results[i]["out"]) for i in range(N_CORES)], axis=0)
    return out.reshape(x.shape).astype(np.float32)


def run_traced(x, weights, trace_dir):
    """test.py only: run with NTFF tracing, artifacts into trace_dir."""
    return run_bass_kernel_spmd(
        _get_nc(), _in_maps(x, weights), list(range(N_CORES)),
        trace=True, tmpdir=trace_dir)


if __name__ == "__main__":
    xs = np.random.randn(4, 2048, D).astype(np.float32)
    ws = (np.broadcast_to(np.eye(BS, dtype=np.float32), (16, BS, BS))
          + 0.02 * np.random.randn(16, BS, BS).astype(np.float32))
    o = kernel(xs, ws)
    print("kernel ran, out shape", o.shape, o.dtype)
